# revision 1
# baseline (speedup 1.0000x reference)
"""Trainium2 Bass kernel for ChannelAwareAttentionModule.

Reference computation (per sample b, x: (256, 4096) = (C, H*W)):
    g     = relu(BN(Wg  @ x))                  (128, 4096)
    theta = relu(BN(Wth @ x))                  (128, 4096)
    phi   = relu(BN(Wph @ x))                  (128, 4096)
    f     = softmax(theta @ phi.T, axis=-1)    (128, 128)
    y     = f @ g                              (128, 4096)
    z     = y.T.reshape(128, 4096)             (torch permute+view scramble)
    out   = BN(Ww @ z) + x                     (256, 4096)

Sharding: pure data-parallel, 2 samples per core on 8 cores.

Kernel-level transformations:
  * BN folded into conv weights/biases on the host; all operands cast to
    fp16 on the host (fp32 accumulation in PSUM throughout).
  * Final-proj bias folded into x (x_adj = x + bw), projection biases
    compensated; the residual (+ x_adj) is added inside the final
    matmul's PSUM group via an identity-weight matmul, so the final
    evacuation is a plain copy.
  * theta/phi are computed directly in n-transposed layout (needed by the
    scores matmul); their (free-dim-varying) biases enter via rank-1
    K=1 matmuls that also open the PSUM banks (start=True).
  * The permute+view scramble z[c', q*128+r] = y[r, 32c'+q] is realized by
    a stride-32 lhsT access on g:
    z[:, q*128:(q+1)*128] = matmul(lhsT=g[:, q::32], rhs=f.T)
    so no transpose of y is ever materialized.
"""

from contextlib import ExitStack

import numpy as np

import concourse.bacc as bacc
import concourse.mybir as mybir
from concourse import tile
from concourse.bass_utils import run_bass_kernel_spmd

F32 = mybir.dt.float32
F16 = mybir.dt.float16
AF = mybir.ActivationFunctionType
ALU = mybir.AluOpType
AX = mybir.AxisListType

NCORES = 8
B, C, CI, N = 16, 256, 128, 4096
BPC = B // NCORES  # samples per core
NQ = N // 128  # 32 column blocks of 128
EPS = 1e-5


def _build_nc():
    nc = bacc.Bacc("TRN2", target_bir_lowering=False, debug=False, num_devices=NCORES)

    x_d = nc.dram_tensor("x", [BPC, C, N], F16, kind="ExternalInput")
    wall_d = nc.dram_tensor("wall", [128, 1792], F16, kind="ExternalInput")
    bg_d = nc.dram_tensor("bg", [CI, 1], F32, kind="ExternalInput")
    out_d = nc.dram_tensor("out", [BPC, C, N], F16, kind="ExternalOutput")

    with tile.TileContext(nc) as tc, ExitStack() as ctx:
        wpool = ctx.enter_context(tc.tile_pool(name="wts", bufs=1))
        xpool = ctx.enter_context(tc.tile_pool(name="xp", bufs=8 * BPC))
        tppool = ctx.enter_context(tc.tile_pool(name="tp", bufs=BPC))
        gpool = ctx.enter_context(tc.tile_pool(name="gp", bufs=BPC))
        zpool = ctx.enter_context(tc.tile_pool(name="zp", bufs=BPC))
        smpool = ctx.enter_context(tc.tile_pool(name="sm", bufs=BPC))
        opool = ctx.enter_context(tc.tile_pool(name="ost", bufs=8))
        # PSUM: big pool (2-bank tiles) for thph/g/z/scores, 1-bank pool for fin
        ps_big = ctx.enter_context(tc.tile_pool(name="ps_big", bufs=2, space="PSUM"))
        ps_sml = ctx.enter_context(tc.tile_pool(name="ps_sml", bufs=2, space="PSUM"))
        ps_fin = ctx.enter_context(tc.tile_pool(name="ps_fin", bufs=2, space="PSUM"))

        # --- replicated constants (single packed DMA + small bg) ---
        w_all = wpool.tile([128, 1792], F16, tag="w_all")
        b_g = wpool.tile([CI, 1], F32, tag="b_g")
        nc.sync.dma_start(w_all[:, :], wall_d[:])
        nc.sync.dma_start(b_g[:, 0:1], bg_d[:])
        w_g = w_all[:, 0:256]
        w_tp = w_all[:, 256:768]
        w_w = w_all[:, 768:1024]
        idf = w_all[:, 1024:1152]
        b_tp = w_all[0:1, 1152:1664]
        ones1 = w_all[0:1, 1664:1792]

        # evac engine round-robin (ACT / DVE)
        state = {"i": 0}

        def pick():
            state["i"] += 1
            return state["i"] % 2 == 0

        xs = {}
        g_sb = {}
        thph = {}

        # ================= phase 1: x loads + projections =================
        for b in range(BPC):
            # x: 2 k-chunks x 2 n-halves of (128, 2048) fp16
            xs[b] = [
                [
                    xpool.tile([128, 1024], F16, tag="x", name=f"x_{b}_{k}_{j}")
                    for j in range(4)
                ]
                for k in range(2)
            ]
            for j in range(4):
                for k in range(2):
                    if b == 0 and j == 0:
                        # split the first chunk so the first matmuls start sooner
                        for s2 in range(2):
                            nc.sync.dma_start(
                                xs[b][k][j][:, 512 * s2 : 512 * (s2 + 1)],
                                x_d[b, 128 * k : 128 * (k + 1),
                                    512 * s2 : 512 * (s2 + 1)],
                            )
                    else:
                        nc.sync.dma_start(
                            xs[b][k][j][:, :],
                            x_d[b, 128 * k : 128 * (k + 1), 1024 * j : 1024 * (j + 1)],
                        )

            thph[b] = tppool.tile([128, NQ * 256], F16, tag="thph", name=f"thph_{b}")
            g_sb[b] = gpool.tile([128, N], F16, tag="g_sb", name=f"g_sb_{b}")

            # theta^T/phi^T projection: psum (128,1024) = 4 q-blocks (2 banks).
            # Each bank is opened by a rank-1 bias matmul (start=True, 512 cols);
            # the projection matmuls then accumulate.
            for t in range(8):
                pt = ps_big.tile([128, 1024], F32, tag="ps_big", name=f"pt_{b}_{t}")
                for half in range(2):
                    nc.tensor.matmul(
                        pt[:, 512 * half : 512 * (half + 1)],
                        ones1,
                        b_tp,
                        start=True,
                        stop=False,
                        skip_group_check=True,
                    )
                for h in range(4):
                    q = 4 * t + h
                    xj, xc = divmod(q * 128, 1024)
                    for k in range(2):
                        nc.tensor.matmul(
                            pt[:, 256 * h : 256 * (h + 1)],
                            xs[b][k][xj][:, xc : xc + 128],
                            w_tp[:, 256 * k : 256 * (k + 1)],
                            start=False,
                            stop=(h == 3 and k == 1),
                            skip_group_check=True,
                        )
                dst = thph[b][:, 1024 * t : 1024 * (t + 1)]
                if pick():
                    nc.scalar.activation(dst, pt[:, :], AF.Relu)
                else:
                    nc.vector.tensor_scalar(dst, pt[:, :], 0.0, None, ALU.max)

            # g projection: psum (128,1024) = 2 n-chunks of 512 (2 banks)
            for t in range(4):
                pg = ps_big.tile([128, 1024], F32, tag="ps_big", name=f"pg_{b}_{t}")
                for half in range(2):
                    j = 2 * t + half
                    xj, xc = divmod(j * 512, 1024)
                    for k in range(2):
                        nc.tensor.matmul(
                            pg[:, 512 * half : 512 * (half + 1)],
                            w_g[:, 128 * k : 128 * (k + 1)],
                            xs[b][k][xj][:, xc : xc + 512],
                            start=(k == 0),
                            stop=(k == 1),
                            skip_group_check=True,
                        )
                dst = g_sb[b][:, 1024 * t : 1024 * (t + 1)]
                if pick():
                    nc.scalar.activation(dst, pg[:, :], AF.Relu, bias=b_g[:, 0:1])
                else:
                    nc.vector.tensor_scalar(
                        dst, pg[:, :], b_g[:, 0:1], 0.0, ALU.add, ALU.max
                    )

        # ================= phase 2: attention + output =================
        for b in range(BPC):
            # scores: s[c,d] accumulated over 32 q-blocks (fp16, N=128)
            ps_s = ps_sml.tile([128, 128], F32, tag="ps_sml", name=f"ps_s_{b}")
            for q in range(NQ):
                nc.tensor.matmul(
                    ps_s[:, :],
                    thph[b][:, 256 * q : 256 * q + 128],
                    thph[b][:, 256 * q + 128 : 256 * (q + 1)],
                    start=(q == 0),
                    stop=(q == NQ - 1),
                    skip_group_check=True,
                )
            negmax = smpool.tile([128, 1], F32, tag="negmax", name=f"negmax_{b}")
            nc.vector.reduce_max(negmax[:, :], ps_s[:, :], axis=AX.X, negate=True)
            e_sb = smpool.tile([128, 128], F16, tag="e_sb", name=f"e_sb_{b}")
            sumex = smpool.tile([128, 1], F32, tag="sumex", name=f"sumex_{b}")
            nc.scalar.activation(
                e_sb[:, :], ps_s[:, :], AF.Exp, bias=negmax[:, :], accum_out=sumex[:, :]
            )
            rs = smpool.tile([128, 1], F32, tag="rs", name=f"rs_{b}")
            nc.vector.reciprocal(rs[:, :], sumex[:, :])
            f_sb = smpool.tile([128, 128], F16, tag="f_sb", name=f"f_sb_{b}")
            nc.scalar.activation(f_sb[:, :], e_sb[:, :], AF.Copy, scale=rs[:, :])
            ps_t = ps_sml.tile([128, 128], F16, tag="ps_sml", name=f"ps_t_{b}")
            nc.tensor.transpose(ps_t[:, :], f_sb[:, :], idf[:, :])
            fT = smpool.tile([128, 128], F16, tag="fT", name=f"fT_{b}")
            nc.vector.tensor_copy(fT[:, :], ps_t[:, :])

            z_sb = zpool.tile([128, N], F16, tag="z_sb", name=f"z_sb_{b}")
            g_v = g_sb[b][:, :].rearrange("p (c q) -> p q c", q=NQ)  # stride-32 view
            # z: psum (128,1024) = 8 q-blocks; each matmul opens its own
            # pending-zero region (order-independent; sim group check skipped)
            for t in range(4):
                pz = ps_big.tile([128, 1024], F32, tag="ps_big", name=f"pz_{b}_{t}")
                for tq in range(8):
                    q = 8 * t + tq
                    nc.tensor.matmul(
                        pz[:, 128 * tq : 128 * (tq + 1)],
                        g_v[:, q, :],
                        fT[:, :],
                        start=True,
                        stop=True,
                        skip_group_check=True,
                    )
                for zh in range(2):
                    dst = z_sb[:, 1024 * t + 512 * zh : 1024 * t + 512 * (zh + 1)]
                    srcp = pz[:, 512 * zh : 512 * (zh + 1)]
                    if pick():
                        nc.scalar.copy(dst, srcp)
                    else:
                        nc.vector.tensor_copy(dst, srcp)

                # final for the 2 n-chunks covered by this z tile
                otb = opool.tile([128, 2, 1024], F16, tag="ost", name=f"ot_{b}_{t}")
                for half in range(2):
                    jj = 2 * t + half
                    ot = otb[:, :, 512 * half : 512 * (half + 1)]
                    for h in range(2):
                        pf = ps_fin.tile(
                            [128, 512], F32, tag="ps_fin", name=f"pf_{b}_{jj}_{h}"
                        )
                        xj, xc = divmod(jj * 512, 1024)
                        if pick():
                            # residual via identity matmul, ACT copy out
                            nc.tensor.matmul(
                                pf[:, :],
                                idf,
                                xs[b][h][xj][:, xc : xc + 512],
                                start=True,
                                stop=False,
                                skip_group_check=True,
                            )
                            nc.tensor.matmul(
                                pf[:, :],
                                w_w[:, 128 * h : 128 * (h + 1)],
                                z_sb[:, 512 * jj : 512 * (jj + 1)],
                                start=False,
                                stop=True,
                                skip_group_check=True,
                            )
                            nc.scalar.copy(ot[:, h, :], pf[:, :])
                        else:
                            # residual fused into the DVE evacuation
                            nc.tensor.matmul(
                                pf[:, :],
                                w_w[:, 128 * h : 128 * (h + 1)],
                                z_sb[:, 512 * jj : 512 * (jj + 1)],
                                start=True,
                                stop=True,
                                skip_group_check=True,
                            )
                            nc.vector.scalar_tensor_tensor(
                                ot[:, h, :],
                                pf[:, :],
                                0.0,
                                xs[b][h][xj][:, xc : xc + 512],
                                ALU.add,
                                ALU.add,
                            )
                # one DMA for both output halves (256 rows x 1024 cols)
                nc.sync.dma_start(
                    out_d[b, :, 1024 * t : 1024 * (t + 1)].rearrange(
                        "(h p) n -> p h n", h=2
                    ),
                    otb[:, :, :],
                )

    nc.compile()
    return nc


_CACHE = {}


def _prepare(inputs):
    """Fold BN into weights/biases and build per-core input maps."""

    def fold(w, bias, gamma, beta, mean, var):
        inv = gamma / np.sqrt(var + EPS)
        return (w * inv[:, None]).astype(np.float32), (
            beta + (bias - mean) * inv
        ).astype(np.float32)

    Wg, bg = fold(
        inputs["g_w"], inputs["g_b"], inputs["g_gamma"], inputs["g_beta"],
        inputs["g_mean"], inputs["g_var"],
    )
    Wth, bth = fold(
        inputs["th_w"], inputs["th_b"], inputs["th_gamma"], inputs["th_beta"],
        inputs["th_mean"], inputs["th_var"],
    )
    Wph, bph = fold(
        inputs["ph_w"], inputs["ph_b"], inputs["ph_gamma"], inputs["ph_beta"],
        inputs["ph_mean"], inputs["ph_var"],
    )
    Ww, bw = fold(
        inputs["w_w"], inputs["w_b"], inputs["w_gamma"], inputs["w_beta"],
        inputs["w_mean"], inputs["w_var"],
    )

    # x_adj = x + bw (per out-channel); compensate projection biases.
    x = np.asarray(inputs["x"], dtype=np.float32).reshape(B, C, N)
    x_adj = (x + bw[None, :, None]).astype(np.float16)
    bg_a = bg - Wg @ bw
    bth_a = bth - Wth @ bw
    bph_a = bph - Wph @ bw

    WgT = np.ascontiguousarray(Wg.T)  # (256, 128)
    wg_host = np.concatenate([WgT[0:128], WgT[128:256]], axis=1)  # (128, 256)
    WtpT = np.concatenate([Wth.T, Wph.T], axis=1)  # (256, 256)
    wtp_host = np.concatenate([WtpT[0:128], WtpT[128:256]], axis=1)  # (128, 512)
    btp_host = np.concatenate([bth_a, bph_a, bth_a, bph_a]).reshape(1, 512)
    ww_host = np.ascontiguousarray(Ww.T)  # (128, 256)

    wall = np.zeros((128, 1792), dtype=np.float16)
    wall[:, 0:256] = wg_host
    wall[:, 256:768] = wtp_host
    wall[:, 768:1024] = ww_host
    wall[:, 1024:1152] = np.eye(128, dtype=np.float16)
    wall[0, 1152:1664] = btp_host[0]
    wall[0, 1664:1792] = 1.0
    consts = {
        "wall": wall,
        "bg": np.ascontiguousarray(bg_a.reshape(CI, 1), dtype=np.float32),
    }
    in_maps = []
    for i in range(NCORES):
        m = dict(consts)
        m["x"] = np.ascontiguousarray(x_adj[BPC * i : BPC * (i + 1)])
        in_maps.append(m)
    return in_maps


def _get_nc():
    if "nc" not in _CACHE:
        _CACHE["nc"] = _build_nc()
    return _CACHE["nc"]


def run(inputs, **kw):
    """Run on hardware; returns (full_output, BassKernelResults)."""
    nc = _get_nc()
    in_maps = _prepare(inputs)
    res = run_bass_kernel_spmd(nc, in_maps, list(range(NCORES)), **kw)
    out = np.concatenate(
        [
            np.asarray(res.results[i]["out"], dtype=np.float32).reshape(BPC, C, 64, 64)
            for i in range(NCORES)
        ],
        axis=0,
    )
    return np.ascontiguousarray(out), res


def kernel(**inputs):
    out, _ = run(inputs)
    return out



# revision 4
# speedup vs baseline: 1.1076x; 1.1076x over previous
"""Trainium2 Bass kernel for ChannelAwareAttentionModule.

Reference computation (per sample b, x: (256, 4096) = (C, H*W)):
    g     = relu(BN(Wg  @ x))                  (128, 4096)
    theta = relu(BN(Wth @ x))                  (128, 4096)
    phi   = relu(BN(Wph @ x))                  (128, 4096)
    f     = softmax(theta @ phi.T, axis=-1)    (128, 128)
    y     = f @ g                              (128, 4096)
    z     = y.T.reshape(128, 4096)             (torch permute+view scramble)
    out   = BN(Ww @ z) + x                     (256, 4096)

Sharding: pure data-parallel, 2 samples per core on 8 cores.

Kernel-level transformations:
  * BN folded into conv weights/biases on the host; all operands cast to
    fp16 on the host (fp32 accumulation in PSUM throughout).
  * Final-proj bias folded into x (x_adj = x + bw), projection biases
    compensated; the residual (+ x_adj) is added during PSUM evacuation
    via scalar_tensor_tensor on DVE/GPSIMD.
  * theta/phi are computed directly in n-transposed layout (needed by the
    scores matmul); their (free-dim-varying) biases enter via rank-1
    K=1 matmuls that also open the PSUM banks (start=True).  The two
    rank-1 matmuls per PSUM pair are packed into disjoint PE row groups
    (tile_position) so they run concurrently.
  * The permute+view scramble z[c', q*128+r] = y[r, 32c'+q] is realized by
    a stride-32 lhsT access on g:
    z[:, q*128:(q+1)*128] = matmul(lhsT=g[:, q::32], rhs=f.T)
    so no transpose of y is ever materialized.
  * A burst of dummy matmuls on zeroed scratch runs during the initial
    DMA window so the PE HAM clock-gate un-throttles (1.2 -> 2.4 GHz)
    before real matmuls start.
  * Per-sample software pipeline: sample 0's attention+output overlaps
    sample 1's projections; output DMAs overlap compute.
"""

from contextlib import ExitStack

import numpy as np

import concourse.bacc as bacc
import concourse.mybir as mybir
from concourse import tile
from concourse.bass_utils import run_bass_kernel_spmd

F32 = mybir.dt.float32
F16 = mybir.dt.float16
AF = mybir.ActivationFunctionType
ALU = mybir.AluOpType
AX = mybir.AxisListType

NCORES = 8
B, C, CI, N = 16, 256, 128, 4096
BPC = B // NCORES  # samples per core
NQ = N // 128  # 32 column blocks of 128
EPS = 1e-5
NWARM = 12  # HAM warm-up matmuls


def _build_nc():
    nc = bacc.Bacc("TRN2", target_bir_lowering=False, debug=False, num_devices=NCORES)

    x_d = nc.dram_tensor("x", [BPC, C, N], F16, kind="ExternalInput")
    wtp_d = nc.dram_tensor("wtp", [128, 512], F16, kind="ExternalInput")
    bt4_d = nc.dram_tensor("bt4", [64, 640], F16, kind="ExternalInput")
    wr_d = nc.dram_tensor("wr", [128, 640], F16, kind="ExternalInput")
    bg_d = nc.dram_tensor("bg", [CI, 1], F32, kind="ExternalInput")
    out_d = nc.dram_tensor("out", [BPC, C, N], F16, kind="ExternalOutput")

    with tile.TileContext(nc) as tc, ExitStack() as ctx:
        wpool = ctx.enter_context(tc.tile_pool(name="wts", bufs=1))
        xpool = ctx.enter_context(tc.tile_pool(name="xp", bufs=BPC))
        tpool = ctx.enter_context(tc.tile_pool(name="thp", bufs=BPC))
        ppool = ctx.enter_context(tc.tile_pool(name="php", bufs=BPC))
        gpool = ctx.enter_context(tc.tile_pool(name="gp", bufs=BPC))
        zpool = ctx.enter_context(tc.tile_pool(name="zp", bufs=BPC))
        smpool = ctx.enter_context(tc.tile_pool(name="sm", bufs=BPC))
        opool = ctx.enter_context(tc.tile_pool(name="ost", bufs=4 * BPC))
        ps_big = ctx.enter_context(tc.tile_pool(name="ps_big", bufs=2, space="PSUM"))
        ps_sml = ctx.enter_context(tc.tile_pool(name="ps_sml", bufs=2, space="PSUM"))
        ps_fin = ctx.enter_context(tc.tile_pool(name="ps_fin", bufs=2, space="PSUM"))

        # --- constants + scratch ---
        wtp_sb = wpool.tile([128, 512], F16, tag="wtp")
        bt4_sb = wpool.tile([64, 640], F16, tag="bt4")
        wr_sb = wpool.tile([128, 640], F16, tag="wr")
        bg_sb = wpool.tile([CI, 1], F32, tag="bg")
        scr = wpool.tile([128, 512], F16, tag="scr")

        # warm-up: run dummy matmuls during the input-DMA window so the
        # HAM clock gate reaches 8/8 before the first real matmul.
        nc.vector.memset(scr[:, :], 0.0)
        ps_w = ps_fin.tile([128, 512], F32, tag="ps_fin", name="ps_warm")
        for _ in range(NWARM):
            nc.tensor.matmul(
                ps_w[:, :], scr[:, 0:128], scr[:, 0:512],
                start=True, stop=True, skip_group_check=True,
            )

        # --- input DMAs (issue order == landing order) ---
        nc.sync.dma_start(bt4_sb[:, :], bt4_d[:])
        nc.sync.dma_start(wtp_sb[:, :], wtp_d[:])
        xs = {}
        for b in range(BPC):
            xs[b] = xpool.tile([128, 2, N], F16, tag="x", name=f"x_{b}")
        # sample 0: first 1024 cols in quarters so compute starts early
        for s in range(2):
            for k in range(2):
                nc.sync.dma_start(
                    xs[0][:, k, 512 * s : 512 * (s + 1)],
                    x_d[0, 128 * k : 128 * (k + 1), 512 * s : 512 * (s + 1)],
                )
        nc.sync.dma_start(wr_sb[:, :], wr_d[:])
        nc.sync.dma_start(bg_sb[:, 0:1], bg_d[:])
        for j in range(1, 4):
            nc.sync.dma_start(
                xs[0][:, :, 1024 * j : 1024 * (j + 1)],
                x_d[0, :, 1024 * j : 1024 * (j + 1)].rearrange("(k p) n -> p k n", k=2),
            )
        for m in range(2):
            nc.sync.dma_start(
                xs[1][:, :, 2048 * m : 2048 * (m + 1)],
                x_d[1, :, 2048 * m : 2048 * (m + 1)].rearrange("(k p) n -> p k n", k=2),
            )

        w_g = wr_sb[:, 0:256]
        w_w = wr_sb[:, 256:512]
        idf = wr_sb[:, 512:640]
        btp0 = bt4_sb[0:1, 0:512]
        ones0 = bt4_sb[0:1, 512:640]
        btp1 = bt4_sb[32:33, 0:512]
        ones1 = bt4_sb[32:33, 512:640]

        # --- evacuation-engine load balancer (GPSIMD cannot read PSUM) ---
        busy = {"act": 0.0, "dve": 0.0}
        RATE = {"act": 6.1, "dve": 9.5}  # ns per 1024 elems

        def pick(nelem, allowed):
            cost = {e: nelem * RATE[e] / 1024.0 for e in allowed}
            e = min(allowed, key=lambda k: busy[k] + cost[k])
            busy[e] += cost[e]
            return e

        def evac_relu(dst, src, nelem, allowed=("act", "dve")):
            e = pick(nelem, allowed)
            if e == "act":
                nc.scalar.activation(dst, src, AF.Relu)
            else:
                nc.vector.tensor_scalar(dst, src, 0.0, None, ALU.max)

        def evac_relu_bias(dst, src, bias, nelem, allowed=("act", "dve")):
            e = pick(nelem, allowed)
            if e == "act":
                nc.scalar.activation(dst, src, AF.Relu, bias=bias)
            else:
                nc.vector.tensor_scalar(dst, src, bias, 0.0, ALU.add, ALU.max)

        def evac_copy(dst, src, nelem, allowed=("act", "dve")):
            e = pick(nelem, allowed)
            if e == "act":
                nc.scalar.copy(dst, src)
            else:
                nc.vector.tensor_copy(dst, src)

        th_sb, ph_sb, g_sb, z_sb, f_soft, fT_sb = {}, {}, {}, {}, {}, {}
        for b in range(BPC):
            th_sb[b] = tpool.tile([128, N], F16, tag="th", name=f"th_{b}")
            ph_sb[b] = ppool.tile([128, N], F16, tag="ph", name=f"ph_{b}")
            g_sb[b] = gpool.tile([128, N], F16, tag="g", name=f"g_{b}")
            z_sb[b] = zpool.tile([128, N], F16, tag="z", name=f"z_{b}")

        def proj_thph(b, trange):
            # psum (128,1024) per t = 4 q-blocks of [theta(128) | phi(128)].
            # The 2 rank-1 bias matmuls (one per 512-col bank) run in
            # disjoint PE row groups -> concurrent.
            for t in trange:
                pt = ps_big.tile([128, 1024], F32, tag="ps_big", name=f"pt_{b}_{t}")
                nc.tensor.matmul(
                    pt[:, 0:512], ones0, btp0,
                    start=True, stop=False, skip_group_check=True,
                    tile_position=(0, 0),
                )
                nc.tensor.matmul(
                    pt[:, 512:1024], ones1, btp1,
                    start=True, stop=False, skip_group_check=True,
                    tile_position=(32, 0),
                )
                for h in range(4):
                    q = 4 * t + h
                    for k in range(2):
                        nc.tensor.matmul(
                            pt[:, 256 * h : 256 * (h + 1)],
                            xs[b][:, k, 128 * q : 128 * (q + 1)],
                            wtp_sb[:, 256 * k : 256 * (k + 1)],
                            start=False,
                            stop=(h == 3 and k == 1),
                            skip_group_check=True,
                        )
                ptv = pt[:, :].rearrange("p (h x) -> p h x", h=4)
                thv = th_sb[b][:, 512 * t : 512 * (t + 1)].rearrange(
                    "p (h x) -> p h x", h=4
                )
                phv = ph_sb[b][:, 512 * t : 512 * (t + 1)].rearrange(
                    "p (h x) -> p h x", h=4
                )
                evac_relu(thv, ptv[:, :, 0:128], 65536)
                evac_relu(phv, ptv[:, :, 128:256], 65536)

        def proj_g(b):
            for t in range(4):
                pg = ps_big.tile([128, 1024], F32, tag="ps_big", name=f"pg_{b}_{t}")
                for half in range(2):
                    j = 2 * t + half
                    for k in range(2):
                        nc.tensor.matmul(
                            pg[:, 512 * half : 512 * (half + 1)],
                            w_g[:, 128 * k : 128 * (k + 1)],
                            xs[b][:, k, 512 * j : 512 * (j + 1)],
                            start=(k == 0),
                            stop=(k == 1),
                            skip_group_check=True,
                        )
                evac_relu_bias(
                    g_sb[b][:, 1024 * t : 1024 * (t + 1)], pg[:, :], bg_sb[:, 0:1],
                    131072,
                )

        def scores(b):
            ps_s = ps_sml.tile([128, 128], F32, tag="ps_sml", name=f"ps_s_{b}")
            for q in range(NQ):
                nc.tensor.matmul(
                    ps_s[:, :],
                    th_sb[b][:, 128 * q : 128 * (q + 1)],
                    ph_sb[b][:, 128 * q : 128 * (q + 1)],
                    start=(q == 0),
                    stop=(q == NQ - 1),
                    skip_group_check=True,
                )
            f_soft[b] = ps_s

        def softmax_ops(b):
            ps_s = f_soft[b]
            negmax = smpool.tile([128, 1], F32, tag="negmax", name=f"negmax_{b}")
            nc.vector.reduce_max(negmax[:, :], ps_s[:, :], axis=AX.X, negate=True)
            e_sb = smpool.tile([128, 128], F16, tag="e_sb", name=f"e_sb_{b}")
            sumex = smpool.tile([128, 1], F32, tag="sumex", name=f"sumex_{b}")
            nc.scalar.activation(
                e_sb[:, :], ps_s[:, :], AF.Exp, bias=negmax[:, :], accum_out=sumex[:, :]
            )
            rs = smpool.tile([128, 1], F32, tag="rs", name=f"rs_{b}")
            nc.vector.reciprocal(rs[:, :], sumex[:, :])
            f_sb = smpool.tile([128, 128], F16, tag="f_sb", name=f"f_sb_{b}")
            nc.scalar.activation(f_sb[:, :], e_sb[:, :], AF.Copy, scale=rs[:, :])
            f_soft[b] = f_sb

        def transpose_f(b):
            ps_t = ps_sml.tile([128, 128], F16, tag="ps_sml", name=f"ps_t_{b}")
            nc.tensor.transpose(ps_t[:, :], f_soft[b][:, :], idf[:, :])
            fT = smpool.tile([128, 128], F16, tag="fT", name=f"fT_{b}")
            nc.vector.tensor_copy(fT[:, :], ps_t[:, :])
            fT_sb[b] = fT

        def zfin(b, trange):
            g_v = g_sb[b][:, :].rearrange("p (c q) -> p q c", q=NQ)  # stride-32 view
            for t in trange:
                pz = ps_big.tile([128, 1024], F32, tag="ps_big", name=f"pz_{b}_{t}")
                for tq in range(8):
                    q = 8 * t + tq
                    nc.tensor.matmul(
                        pz[:, 128 * tq : 128 * (tq + 1)],
                        g_v[:, q, :],
                        fT_sb[b][:, :],
                        start=True,
                        stop=True,
                        skip_group_check=True,
                    )
                z = z_sb[b]
                for zh in range(2):
                    evac_copy(
                        z[:, 1024 * t + 512 * zh : 1024 * t + 512 * (zh + 1)],
                        pz[:, 512 * zh : 512 * (zh + 1)],
                        65536,
                    )
                otb = opool.tile([128, 2, 1024], F16, tag="ost", name=f"ot_{b}_{t}")
                for half in range(2):
                    jj = 2 * t + half
                    for h in range(2):
                        pf = ps_fin.tile(
                            [128, 512], F32, tag="ps_fin", name=f"pf_{b}_{jj}_{h}"
                        )
                        ot = otb[:, h, 512 * half : 512 * (half + 1)]
                        xres = xs[b][:, h, 512 * jj : 512 * (jj + 1)]
                        e = pick(65536, ("act", "dve"))
                        if e == "act":
                            # residual via identity matmul, ACT copy out
                            nc.tensor.matmul(
                                pf[:, :], idf, xres,
                                start=True, stop=False, skip_group_check=True,
                            )
                            nc.tensor.matmul(
                                pf[:, :],
                                w_w[:, 128 * h : 128 * (h + 1)],
                                z[:, 512 * jj : 512 * (jj + 1)],
                                start=False, stop=True, skip_group_check=True,
                            )
                            nc.scalar.copy(ot, pf[:, :])
                        else:
                            # residual fused into the DVE evacuation
                            nc.tensor.matmul(
                                pf[:, :],
                                w_w[:, 128 * h : 128 * (h + 1)],
                                z[:, 512 * jj : 512 * (jj + 1)],
                                start=True, stop=True, skip_group_check=True,
                            )
                            nc.vector.scalar_tensor_tensor(
                                ot, pf[:, :], 0.0, xres, ALU.add, ALU.add
                            )
                nc.sync.dma_start(
                    out_d[b, :, 1024 * t : 1024 * (t + 1)].rearrange(
                        "(h p) n -> p h n", h=2
                    ),
                    otb[:, :, :],
                )

        # --- software pipeline over the 2 samples ---
        proj_thph(0, range(8))
        proj_g(0)
        scores(0)
        softmax_ops(0)
        proj_thph(1, range(2))     # covers sample-0 softmax latency
        transpose_f(0)
        proj_thph(1, range(2, 3))  # covers sample-0 fT copy latency
        zfin(0, range(3))
        proj_thph(1, range(3, 8))
        proj_g(1)
        scores(1)
        softmax_ops(1)
        zfin(0, range(3, 4))       # covers sample-1 softmax latency
        transpose_f(1)
        zfin(1, range(4))

    nc.compile()
    return nc


_CACHE = {}


def _prepare(inputs):
    """Fold BN into weights/biases and build per-core input maps."""

    def fold(w, bias, gamma, beta, mean, var):
        inv = gamma / np.sqrt(var + EPS)
        return (w * inv[:, None]).astype(np.float32), (
            beta + (bias - mean) * inv
        ).astype(np.float32)

    Wg, bg = fold(
        inputs["g_w"], inputs["g_b"], inputs["g_gamma"], inputs["g_beta"],
        inputs["g_mean"], inputs["g_var"],
    )
    Wth, bth = fold(
        inputs["th_w"], inputs["th_b"], inputs["th_gamma"], inputs["th_beta"],
        inputs["th_mean"], inputs["th_var"],
    )
    Wph, bph = fold(
        inputs["ph_w"], inputs["ph_b"], inputs["ph_gamma"], inputs["ph_beta"],
        inputs["ph_mean"], inputs["ph_var"],
    )
    Ww, bw = fold(
        inputs["w_w"], inputs["w_b"], inputs["w_gamma"], inputs["w_beta"],
        inputs["w_mean"], inputs["w_var"],
    )

    # x_adj = x + bw (per out-channel); compensate projection biases.
    x = np.asarray(inputs["x"], dtype=np.float32).reshape(B, C, N)
    x_adj = (x + bw[None, :, None]).astype(np.float16)
    bg_a = bg - Wg @ bw
    bth_a = bth - Wth @ bw
    bph_a = bph - Wph @ bw

    WgT = np.ascontiguousarray(Wg.T)  # (256, 128)
    wg_host = np.concatenate([WgT[0:128], WgT[128:256]], axis=1)  # (128, 256)
    WtpT = np.concatenate([Wth.T, Wph.T], axis=1)  # (256, 256)
    wtp_host = np.concatenate([WtpT[0:128], WtpT[128:256]], axis=1)  # (128, 512)
    btp_host = np.concatenate([bth_a, bph_a, bth_a, bph_a])  # (512,)
    ww_host = np.ascontiguousarray(Ww.T)  # (128, 256)

    bt4 = np.zeros((64, 640), dtype=np.float16)
    for r in (0, 32):
        bt4[r, 0:512] = btp_host
        bt4[r, 512:640] = 1.0
    wr = np.zeros((128, 640), dtype=np.float16)
    wr[:, 0:256] = wg_host
    wr[:, 256:512] = ww_host
    wr[:, 512:640] = np.eye(128, dtype=np.float16)
    consts = {
        "wtp": wtp_host.astype(np.float16),
        "bt4": bt4,
        "wr": wr,
        "bg": np.ascontiguousarray(bg_a.reshape(CI, 1), dtype=np.float32),
    }
    in_maps = []
    for i in range(NCORES):
        m = dict(consts)
        m["x"] = np.ascontiguousarray(x_adj[BPC * i : BPC * (i + 1)])
        in_maps.append(m)
    return in_maps


def _get_nc():
    if "nc" not in _CACHE:
        _CACHE["nc"] = _build_nc()
    return _CACHE["nc"]


def run(inputs, **kw):
    """Run on hardware; returns (full_output, BassKernelResults)."""
    nc = _get_nc()
    in_maps = _prepare(inputs)
    res = run_bass_kernel_spmd(nc, in_maps, list(range(NCORES)), **kw)
    out = np.concatenate(
        [
            np.asarray(res.results[i]["out"], dtype=np.float32).reshape(BPC, C, 64, 64)
            for i in range(NCORES)
        ],
        axis=0,
    )
    return np.ascontiguousarray(out), res


def kernel(**inputs):
    out, _ = run(inputs)
    return out


# revision 9
# speedup vs baseline: 1.1489x; 1.0373x over previous
"""Trainium2 Bass kernel for ChannelAwareAttentionModule.

Reference computation (per sample b, x: (256, 4096) = (C, H*W)):
    g     = relu(BN(Wg  @ x))                  (128, 4096)
    theta = relu(BN(Wth @ x))                  (128, 4096)
    phi   = relu(BN(Wph @ x))                  (128, 4096)
    f     = softmax(theta @ phi.T, axis=-1)    (128, 128)
    y     = f @ g                              (128, 4096)
    z     = y.T.reshape(128, 4096)             (torch permute+view scramble)
    out   = BN(Ww @ z) + x                     (256, 4096)

Sharding: pure data-parallel, 2 samples per core on 8 cores.

Kernel-level transformations:
  * BN folded into conv weights/biases on the host; all operands cast to
    fp16 on the host (fp32 accumulation in PSUM throughout).
  * Final-proj bias folded into x (x_adj = x + bw), projection biases
    compensated; the residual (+ x_adj) is added during PSUM evacuation
    via scalar_tensor_tensor on DVE/GPSIMD.
  * theta/phi are computed directly in n-transposed layout (needed by the
    scores matmul); their (free-dim-varying) biases enter via rank-1
    K=1 matmuls that also open the PSUM banks (start=True).  The two
    rank-1 matmuls per PSUM pair are packed into disjoint PE row groups
    (tile_position) so they run concurrently.
  * The permute+view scramble z[c', q*128+r] = y[r, 32c'+q] is realized by
    a stride-32 lhsT access on g:
    z[:, q*128:(q+1)*128] = matmul(lhsT=g[:, q::32], rhs=f.T)
    so no transpose of y is ever materialized.
  * A burst of dummy matmuls on zeroed scratch runs during the initial
    DMA window so the PE HAM clock-gate un-throttles (1.2 -> 2.4 GHz)
    before real matmuls start.
  * Per-sample software pipeline: sample 0's attention+output overlaps
    sample 1's projections; output DMAs overlap compute.
"""

from contextlib import ExitStack

import numpy as np

import concourse.bacc as bacc
import concourse.mybir as mybir
from concourse import tile
from concourse.bass_utils import run_bass_kernel_spmd

F32 = mybir.dt.float32
F16 = mybir.dt.float16
AF = mybir.ActivationFunctionType
ALU = mybir.AluOpType
AX = mybir.AxisListType

NCORES = 8
B, C, CI, N = 16, 256, 128, 4096
BPC = B // NCORES  # samples per core
NQ = N // 128  # 32 column blocks of 128
EPS = 1e-5
NWARM = 12  # HAM warm-up matmuls


def _build_nc():
    nc = bacc.Bacc("TRN2", target_bir_lowering=False, debug=False, num_devices=NCORES)

    x_d = nc.dram_tensor("x", [BPC, C, N], F16, kind="ExternalInput")
    wtp_d = nc.dram_tensor("wtp", [128, 512], F16, kind="ExternalInput")
    bt4_d = nc.dram_tensor("bt4", [64, 640], F16, kind="ExternalInput")
    wr_d = nc.dram_tensor("wr", [128, 640], F16, kind="ExternalInput")
    bg_d = nc.dram_tensor("bg", [CI, 1], F32, kind="ExternalInput")
    out_d = nc.dram_tensor("out", [BPC, C, N], F16, kind="ExternalOutput")

    with tile.TileContext(nc) as tc, ExitStack() as ctx:
        wpool = ctx.enter_context(tc.tile_pool(name="wts", bufs=1))
        xpool = ctx.enter_context(tc.tile_pool(name="xp", bufs=BPC))
        tpool = ctx.enter_context(tc.tile_pool(name="thp", bufs=BPC))
        ppool = ctx.enter_context(tc.tile_pool(name="php", bufs=BPC))
        gpool = ctx.enter_context(tc.tile_pool(name="gp", bufs=BPC))
        zpool = ctx.enter_context(tc.tile_pool(name="zp", bufs=BPC))
        smpool = ctx.enter_context(tc.tile_pool(name="sm", bufs=BPC))
        opool = ctx.enter_context(tc.tile_pool(name="ost", bufs=4 * BPC))
        ps_big = ctx.enter_context(tc.tile_pool(name="ps_big", bufs=2, space="PSUM"))
        ps_sml = ctx.enter_context(tc.tile_pool(name="ps_sml", bufs=2, space="PSUM"))
        ps_fin = ctx.enter_context(tc.tile_pool(name="ps_fin", bufs=2, space="PSUM"))

        # --- constants + scratch ---
        wtp_sb = wpool.tile([128, 512], F16, tag="wtp")
        bt4_sb = wpool.tile([64, 640], F16, tag="bt4")
        wr_sb = wpool.tile([128, 640], F16, tag="wr")
        bg_sb = wpool.tile([CI, 1], F32, tag="bg")
        scr = wpool.tile([128, 512], F16, tag="scr")

        # warm-up: run dummy matmuls during the input-DMA window so the
        # HAM clock gate reaches 8/8 before the first real matmul.
        nc.vector.memset(scr[:, :], 0.0)
        ps_w = ps_fin.tile([128, 512], F32, tag="ps_fin", name="ps_warm")
        for _ in range(NWARM):
            nc.tensor.matmul(
                ps_w[:, :], scr[:, 0:128], scr[:, 0:512],
                start=True, stop=True, skip_group_check=True,
            )

        # --- input DMAs (issue order == landing order; match PE consumption) ---
        xs = {}
        for b in range(BPC):
            xs[b] = xpool.tile([128, 2, N], F16, tag="x", name=f"x_{b}")
        nc.sync.dma_start(bt4_sb[:, :], bt4_d[:])
        # sample 0: first 1024 cols in quarters so compute starts early
        for k in range(2):
            nc.sync.dma_start(
                xs[0][:, k, 0:512], x_d[0, 128 * k : 128 * (k + 1), 0:512]
            )
        nc.sync.dma_start(wtp_sb[:, :], wtp_d[:])
        for k in range(2):
            nc.sync.dma_start(
                xs[0][:, k, 512:1024], x_d[0, 128 * k : 128 * (k + 1), 512:1024]
            )
        nc.sync.dma_start(
            xs[0][:, :, 1024:2048],
            x_d[0, :, 1024:2048].rearrange("(k p) n -> p k n", k=2),
        )
        nc.sync.dma_start(wr_sb[:, :], wr_d[:])
        nc.sync.dma_start(bg_sb[:, 0:1], bg_d[:])
        for j in range(2, 4):
            nc.sync.dma_start(
                xs[0][:, :, 1024 * j : 1024 * (j + 1)],
                x_d[0, :, 1024 * j : 1024 * (j + 1)].rearrange("(k p) n -> p k n", k=2),
            )
        for m in range(2):
            nc.sync.dma_start(
                xs[1][:, :, 2048 * m : 2048 * (m + 1)],
                x_d[1, :, 2048 * m : 2048 * (m + 1)].rearrange("(k p) n -> p k n", k=2),
            )

        w_g = wr_sb[:, 0:256]
        w_w = wr_sb[:, 256:512]
        idf = wr_sb[:, 512:640]
        btp0 = bt4_sb[0:1, 0:512]
        ones0 = bt4_sb[0:1, 512:640]
        btp1 = bt4_sb[32:33, 0:512]
        ones1 = bt4_sb[32:33, 512:640]

        # --- evacuation-engine load balancer (GPSIMD cannot read PSUM) ---
        busy = {"act": 0.0, "dve": 0.0}
        RATE = {"act": 10.6, "dve": 10.5}  # ns per 1024 elems (PSUM-read ops)

        def pick(nelem, allowed):
            cost = {e: nelem * RATE[e] / 1024.0 for e in allowed}
            e = min(allowed, key=lambda k: busy[k] + cost[k])
            busy[e] += cost[e]
            return e

        def evac_relu(dst, src, nelem, allowed=("act", "dve")):
            e = pick(nelem, allowed)
            if e == "act":
                nc.scalar.activation(dst, src, AF.Relu)
            else:
                nc.vector.tensor_scalar(dst, src, 0.0, None, ALU.max)

        def evac_relu_bias(dst, src, bias, nelem, allowed=("act", "dve")):
            e = pick(nelem, allowed)
            if e == "act":
                nc.scalar.activation(dst, src, AF.Relu, bias=bias)
            else:
                nc.vector.tensor_scalar(dst, src, bias, 0.0, ALU.add, ALU.max)

        def evac_copy(dst, src, nelem, allowed=("act", "dve")):
            e = pick(nelem, allowed)
            if e == "act":
                nc.scalar.copy(dst, src)
            else:
                nc.vector.tensor_copy(dst, src)

        th_sb, ph_sb, g_sb, z_sb, f_soft, fT_sb = {}, {}, {}, {}, {}, {}
        for b in range(BPC):
            th_sb[b] = tpool.tile([128, N], F16, tag="th", name=f"th_{b}")
            ph_sb[b] = ppool.tile([128, N], F16, tag="ph", name=f"ph_{b}")
            g_sb[b] = gpool.tile([128, N], F16, tag="g", name=f"g_{b}")
            z_sb[b] = zpool.tile([128, N], F16, tag="z", name=f"z_{b}")

        def proj_thph(b, trange):
            # psum (128,1024) per t = 4 q-blocks of [theta(128) | phi(128)].
            # The 2 rank-1 bias matmuls (one per 512-col bank) run in
            # disjoint PE row groups -> concurrent.
            for t in trange:
                pt = ps_big.tile([128, 1024], F32, tag="ps_big", name=f"pt_{b}_{t}")
                nc.tensor.matmul(
                    pt[:, 0:512], ones0, btp0,
                    start=True, stop=False, skip_group_check=True,
                    tile_position=(0, 0),
                )
                nc.tensor.matmul(
                    pt[:, 512:1024], ones1, btp1,
                    start=True, stop=False, skip_group_check=True,
                    tile_position=(32, 0),
                )
                for h in range(4):
                    q = 4 * t + h
                    for k in range(2):
                        nc.tensor.matmul(
                            pt[:, 256 * h : 256 * (h + 1)],
                            xs[b][:, k, 128 * q : 128 * (q + 1)],
                            wtp_sb[:, 256 * k : 256 * (k + 1)],
                            start=False,
                            stop=(h == 3 and k == 1),
                            skip_group_check=True,
                        )
                ptv = pt[:, :].rearrange("p (h x) -> p h x", h=4)
                thv = th_sb[b][:, 512 * t : 512 * (t + 1)].rearrange(
                    "p (h x) -> p h x", h=4
                )
                phv = ph_sb[b][:, 512 * t : 512 * (t + 1)].rearrange(
                    "p (h x) -> p h x", h=4
                )
                evac_relu(thv, ptv[:, :, 0:128], 65536)
                evac_relu(phv, ptv[:, :, 128:256], 65536)

        def proj_g(b):
            for t in range(4):
                pg = ps_big.tile([128, 1024], F32, tag="ps_big", name=f"pg_{b}_{t}")
                for half in range(2):
                    j = 2 * t + half
                    for k in range(2):
                        nc.tensor.matmul(
                            pg[:, 512 * half : 512 * (half + 1)],
                            w_g[:, 128 * k : 128 * (k + 1)],
                            xs[b][:, k, 512 * j : 512 * (j + 1)],
                            start=(k == 0),
                            stop=(k == 1),
                            skip_group_check=True,
                        )
                evac_relu_bias(
                    g_sb[b][:, 1024 * t : 1024 * (t + 1)], pg[:, :], bg_sb[:, 0:1],
                    131072,
                )

        def scores(b):
            ps_s = ps_sml.tile([128, 128], F32, tag="ps_sml", name=f"ps_s_{b}")
            for q in range(NQ):
                nc.tensor.matmul(
                    ps_s[:, :],
                    th_sb[b][:, 128 * q : 128 * (q + 1)],
                    ph_sb[b][:, 128 * q : 128 * (q + 1)],
                    start=(q == 0),
                    stop=(q == NQ - 1),
                    skip_group_check=True,
                )
            f_soft[b] = ps_s

        def softmax_ops(b):
            ps_s = f_soft[b]
            negmax = smpool.tile([128, 1], F32, tag="negmax", name=f"negmax_{b}")
            nc.vector.reduce_max(negmax[:, :], ps_s[:, :], axis=AX.X, negate=True)
            e_sb = smpool.tile([128, 128], F16, tag="e_sb", name=f"e_sb_{b}")
            sumex = smpool.tile([128, 1], F32, tag="sumex", name=f"sumex_{b}")
            nc.scalar.activation(
                e_sb[:, :], ps_s[:, :], AF.Exp, bias=negmax[:, :], accum_out=sumex[:, :]
            )
            rs = smpool.tile([128, 1], F32, tag="rs", name=f"rs_{b}")
            nc.vector.reciprocal(rs[:, :], sumex[:, :])
            f_sb = smpool.tile([128, 128], F16, tag="f_sb", name=f"f_sb_{b}")
            nc.scalar.activation(f_sb[:, :], e_sb[:, :], AF.Copy, scale=rs[:, :])
            f_soft[b] = f_sb

        def transpose_f(b):
            ps_t = ps_sml.tile([128, 128], F16, tag="ps_sml", name=f"ps_t_{b}")
            nc.tensor.transpose(ps_t[:, :], f_soft[b][:, :], idf[:, :])
            fT = smpool.tile([128, 128], F16, tag="fT", name=f"fT_{b}")
            nc.vector.tensor_copy(fT[:, :], ps_t[:, :])
            fT_sb[b] = fT

        def zfin(b, trange, split_out=False):
            g_v = g_sb[b][:, :].rearrange("p (c q) -> p q c", q=NQ)  # stride-32 view
            for t in trange:
                pz = ps_big.tile([128, 1024], F32, tag="ps_big", name=f"pz_{b}_{t}")
                for tq in range(8):
                    q = 8 * t + tq
                    nc.tensor.matmul(
                        pz[:, 128 * tq : 128 * (tq + 1)],
                        g_v[:, q, :],
                        fT_sb[b][:, :],
                        start=True,
                        stop=True,
                        skip_group_check=True,
                    )
                z = z_sb[b]
                for zh in range(2):
                    evac_copy(
                        z[:, 1024 * t + 512 * zh : 1024 * t + 512 * (zh + 1)],
                        pz[:, 512 * zh : 512 * (zh + 1)],
                        65536,
                    )
                otb = opool.tile([128, 2, 1024], F16, tag="ost", name=f"ot_{b}_{t}")
                for half in range(2):
                    jj = 2 * t + half
                    for h in range(2):
                        pf = ps_fin.tile(
                            [128, 512], F32, tag="ps_fin", name=f"pf_{b}_{jj}_{h}"
                        )
                        ot = otb[:, h, 512 * half : 512 * (half + 1)]
                        xres = xs[b][:, h, 512 * jj : 512 * (jj + 1)]
                        e = pick(65536, ("act", "dve"))
                        if e == "act":
                            # residual via identity matmul, ACT copy out
                            nc.tensor.matmul(
                                pf[:, :], idf, xres,
                                start=True, stop=False, skip_group_check=True,
                            )
                            nc.tensor.matmul(
                                pf[:, :],
                                w_w[:, 128 * h : 128 * (h + 1)],
                                z[:, 512 * jj : 512 * (jj + 1)],
                                start=False, stop=True, skip_group_check=True,
                            )
                            nc.scalar.copy(ot, pf[:, :])
                        else:
                            # residual fused into the DVE evacuation
                            nc.tensor.matmul(
                                pf[:, :],
                                w_w[:, 128 * h : 128 * (h + 1)],
                                z[:, 512 * jj : 512 * (jj + 1)],
                                start=True, stop=True, skip_group_check=True,
                            )
                            nc.vector.scalar_tensor_tensor(
                                ot, pf[:, :], 0.0, xres, ALU.add, ALU.add
                            )
                    if split_out:
                        # drain each half as soon as its 2 finals are done
                        nc.sync.dma_start(
                            out_d[
                                b, :, 1024 * t + 512 * half : 1024 * t + 512 * (half + 1)
                            ].rearrange("(h p) n -> p h n", h=2),
                            otb[:, :, 512 * half : 512 * (half + 1)],
                        )
                if not split_out:
                    nc.sync.dma_start(
                        out_d[b, :, 1024 * t : 1024 * (t + 1)].rearrange(
                            "(h p) n -> p h n", h=2
                        ),
                        otb[:, :, :],
                    )

        # --- software pipeline over the 2 samples ---
        proj_thph(0, range(8))
        proj_g(0)
        scores(0)
        softmax_ops(0)
        proj_thph(1, range(2))     # covers sample-0 softmax latency
        transpose_f(0)
        proj_thph(1, range(2, 3))  # covers sample-0 fT copy latency
        # interleave sample-0 attention/output with sample-1 projections so
        # the evacuation engines see a smooth load instead of bursts
        zfin(0, range(0, 1))
        proj_thph(1, range(3, 4))
        zfin(0, range(1, 2))
        proj_thph(1, range(4, 5))
        zfin(0, range(2, 3))
        proj_thph(1, range(5, 8))
        zfin(0, range(3, 4))       # covers sample-1 thph tail evacuation
        scores(1)
        softmax_ops(1)
        proj_g(1)                  # covers sample-1 softmax latency
        transpose_f(1)
        zfin(1, range(4), split_out=True)

    nc.compile()
    return nc


_CACHE = {}


def _prepare(inputs):
    """Fold BN into weights/biases and build per-core input maps."""

    def fold(w, bias, gamma, beta, mean, var):
        inv = gamma / np.sqrt(var + EPS)
        return (w * inv[:, None]).astype(np.float32), (
            beta + (bias - mean) * inv
        ).astype(np.float32)

    Wg, bg = fold(
        inputs["g_w"], inputs["g_b"], inputs["g_gamma"], inputs["g_beta"],
        inputs["g_mean"], inputs["g_var"],
    )
    Wth, bth = fold(
        inputs["th_w"], inputs["th_b"], inputs["th_gamma"], inputs["th_beta"],
        inputs["th_mean"], inputs["th_var"],
    )
    Wph, bph = fold(
        inputs["ph_w"], inputs["ph_b"], inputs["ph_gamma"], inputs["ph_beta"],
        inputs["ph_mean"], inputs["ph_var"],
    )
    Ww, bw = fold(
        inputs["w_w"], inputs["w_b"], inputs["w_gamma"], inputs["w_beta"],
        inputs["w_mean"], inputs["w_var"],
    )

    # x_adj = x + bw (per out-channel); compensate projection biases.
    x = np.asarray(inputs["x"], dtype=np.float32).reshape(B, C, N)
    x_adj = (x + bw[None, :, None]).astype(np.float16)
    bg_a = bg - Wg @ bw
    bth_a = bth - Wth @ bw
    bph_a = bph - Wph @ bw

    WgT = np.ascontiguousarray(Wg.T)  # (256, 128)
    wg_host = np.concatenate([WgT[0:128], WgT[128:256]], axis=1)  # (128, 256)
    WtpT = np.concatenate([Wth.T, Wph.T], axis=1)  # (256, 256)
    wtp_host = np.concatenate([WtpT[0:128], WtpT[128:256]], axis=1)  # (128, 512)
    btp_host = np.concatenate([bth_a, bph_a, bth_a, bph_a])  # (512,)
    ww_host = np.ascontiguousarray(Ww.T)  # (128, 256)

    bt4 = np.zeros((64, 640), dtype=np.float16)
    for r in (0, 32):
        bt4[r, 0:512] = btp_host
        bt4[r, 512:640] = 1.0
    wr = np.zeros((128, 640), dtype=np.float16)
    wr[:, 0:256] = wg_host
    wr[:, 256:512] = ww_host
    wr[:, 512:640] = np.eye(128, dtype=np.float16)
    consts = {
        "wtp": wtp_host.astype(np.float16),
        "bt4": bt4,
        "wr": wr,
        "bg": np.ascontiguousarray(bg_a.reshape(CI, 1), dtype=np.float32),
    }
    in_maps = []
    for i in range(NCORES):
        m = dict(consts)
        m["x"] = np.ascontiguousarray(x_adj[BPC * i : BPC * (i + 1)])
        in_maps.append(m)
    return in_maps


def _get_nc():
    if "nc" not in _CACHE:
        _CACHE["nc"] = _build_nc()
    return _CACHE["nc"]


def run(inputs, **kw):
    """Run on hardware; returns (full_output, BassKernelResults)."""
    nc = _get_nc()
    in_maps = _prepare(inputs)
    res = run_bass_kernel_spmd(nc, in_maps, list(range(NCORES)), **kw)
    out = np.concatenate(
        [
            np.asarray(res.results[i]["out"], dtype=np.float32).reshape(BPC, C, 64, 64)
            for i in range(NCORES)
        ],
        axis=0,
    )
    return np.ascontiguousarray(out), res


def kernel(**inputs):
    out, _ = run(inputs)
    return out
